# revision 1
# baseline (speedup 1.0000x reference)
"""MicroMoE (B=4, T=4096, H=1024, E=8, top-2, SwiGLU F=1024) on 8 TRN2 cores.

Expert-parallel sharding (one expert per NeuronCore):
- The router is replicated on host with jax-on-CPU, using the exact ops the
  reference uses (matmul -> softmax -> top_k -> renormalize), so routing
  decisions and lb_loss match the reference bit-for-bit.
- Dispatch: tokens are gathered per expert on host, padded to a shared
  capacity C (SPMD: all 8 cores run the same program on their own expert's
  tokens + weights).
- Each core runs a dense SwiGLU FFN over its C tokens with fp32r matmuls
  on the PE array (full 1 cycle/row rate, near-fp32 precision), weights
  resident in SBUF:  y = silu(x @ G^T) * (x @ U^T) @ D^T.
- Combine: host scales rows by the gate values and scatter-adds per expert
  (within one expert the token indices are unique, and experts are applied
  in ascending order like the reference's accumulation loop).
"""
import sys
import numpy as np

try:
    import concourse.bass  # noqa: F401
except ImportError:
    sys.path.insert(0, "/opt/trn_rl_repo")

import concourse.bacc as bacc
import concourse.mybir as mybir
import concourse.tile as tile
from concourse import bass_utils

H = 1024          # hidden dim
F = 1024          # expert ffn dim
E = 8             # experts == cores
KTOP = 2          # experts per token
NT = 512          # max tokens per on-chip tile (fp32 moving-dim / PSUM bank)
HB = H // 128
FB = F // 128

f32 = mybir.dt.float32
f32r = mybir.dt.float32r
SILU = mybir.ActivationFunctionType.Silu


def _tile_sizes_for(C: int) -> tuple:
    """Decompose C into token tiles of 256..512 (multiples of 128).

    fp32r needs a moving dim >= 256 for the full PE rate; 512 fp32 outputs
    fill one PSUM bank. A mix of 512/384 tiles covers any C (mult of 128).
    """
    assert C % 128 == 0 and C >= 256
    blocks = C // 128
    n = -(-blocks // 4)
    lo = blocks // n
    hi_cnt = blocks - lo * n
    return (128 * (lo + 1),) * hi_cnt + (128 * lo,) * (n - hi_cnt)


def _build_kernel(tsizes: tuple):
    """Bass program: dense SwiGLU FFN over C tokens, weights SBUF-resident.

    DRAM I/O per core (all fp32 bits; fp32r dtype = PE rounds operands):
      xt [128, HB, C]   xt[p, hb, t] = x[t, hb*128+p]
      wg, wu [128, HB, F]   wg[p, hb, f] = gate_w.T[hb*128+p, f]
      wd [128, FB, H]       wd[p, fb, h] = down_w.T[fb*128+p, h]
      yt [128, HB, C]   yt[p, hb, t] = y[t, hb*128+p]
    """
    C = sum(tsizes)
    nc = bacc.Bacc("TRN2", target_bir_lowering=False, debug=False)
    xt_d = nc.dram_tensor("xt", [128, HB, C], f32r, kind="ExternalInput").ap()
    wg_d = nc.dram_tensor("wg", [128, HB, F], f32r, kind="ExternalInput").ap()
    wu_d = nc.dram_tensor("wu", [128, HB, F], f32r, kind="ExternalInput").ap()
    wd_d = nc.dram_tensor("wd", [128, FB, H], f32r, kind="ExternalInput").ap()
    yt_d = nc.dram_tensor("yt", [128, HB, C], f32, kind="ExternalOutput").ap()

    with tile.TileContext(nc) as tc:
        with (
            tc.tile_pool(name="wpool", bufs=1) as wpool,
            tc.tile_pool(name="io", bufs=2) as io,
            tc.tile_pool(name="ps", bufs=2, space="PSUM") as ps,
        ):
            # per-block weight tiles: first matmuls wait only on the blocks
            # they read, not the whole 12 MB weight load
            wg = [wpool.tile([128, F], f32r, tag=f"wg{i}", name=f"wg{i}")
                  for i in range(HB)]
            wu = [wpool.tile([128, F], f32r, tag=f"wu{i}", name=f"wu{i}")
                  for i in range(HB)]
            wd = [wpool.tile([128, H], f32r, tag=f"wd{i}", name=f"wd{i}")
                  for i in range(FB)]
            # first-needed first: x tile 0, gate weights, then up, then down
            xt0 = io.tile([128, HB, tsizes[0]], f32r, tag="xt", name="xt0")
            nc.sync.dma_start(xt0[:], xt_d[:, :, 0:tsizes[0]])
            for hb in range(HB):
                nc.sync.dma_start(wg[hb][:], wg_d[:, hb, :])
            for hb in range(HB):
                nc.sync.dma_start(wu[hb][:], wu_d[:, hb, :])
            for fb in range(FB):
                nc.sync.dma_start(wd[fb][:], wd_d[:, fb, :])

            t0 = 0
            for t, nt in enumerate(tsizes):
                ts = slice(t0, t0 + nt)
                t0 += nt
                if t == 0:
                    xt = xt0
                else:
                    xt = io.tile([128, HB, nt], f32r, tag="xt")
                    nc.sync.dma_start(xt[:], xt_d[:, :, ts])

                hid = io.tile([128, FB, nt], f32r, tag="hid")
                if t == 0:
                    # gate weights land first; run all G groups before any U
                    # group so the PE starts without waiting for wu
                    for fb in range(FB):
                        psg = ps.tile([128, nt], f32, tag="psg")
                        fc = slice(fb * 128, (fb + 1) * 128)
                        for hb in range(HB):
                            nc.tensor.matmul(
                                psg[:], wg[hb][:, fc], xt[:, hb, :],
                                start=(hb == 0), stop=(hb == HB - 1))
                        nc.scalar.activation(hid[:, fb, :], psg[:], SILU)
                    for fb in range(FB):
                        psu = ps.tile([128, nt], f32, tag="psu")
                        fc = slice(fb * 128, (fb + 1) * 128)
                        for hb in range(HB):
                            nc.tensor.matmul(
                                psu[:], wu[hb][:, fc], xt[:, hb, :],
                                start=(hb == 0), stop=(hb == HB - 1))
                        nc.vector.tensor_mul(hid[:, fb, :], hid[:, fb, :], psu[:])
                else:
                    for fb in range(FB):
                        psg = ps.tile([128, nt], f32, tag="psg")
                        psu = ps.tile([128, nt], f32, tag="psu")
                        fc = slice(fb * 128, (fb + 1) * 128)
                        for hb in range(HB):
                            nc.tensor.matmul(
                                psg[:], wg[hb][:, fc], xt[:, hb, :],
                                start=(hb == 0), stop=(hb == HB - 1))
                        for hb in range(HB):
                            nc.tensor.matmul(
                                psu[:], wu[hb][:, fc], xt[:, hb, :],
                                start=(hb == 0), stop=(hb == HB - 1))
                        nc.scalar.activation(hid[:, fb, :], psg[:], SILU)
                        nc.vector.tensor_mul(hid[:, fb, :], hid[:, fb, :], psu[:])

                ysb = io.tile([128, HB, nt], f32, tag="ysb")
                for ob in range(HB):
                    pso = ps.tile([128, nt], f32, tag="pso")
                    oc = slice(ob * 128, (ob + 1) * 128)
                    for fb in range(FB):
                        nc.tensor.matmul(
                            pso[:], wd[fb][:, oc], hid[:, fb, :],
                            start=(fb == 0), stop=(fb == FB - 1))
                    nc.vector.tensor_copy(ysb[:, ob, :], pso[:])
                    nc.sync.dma_start(yt_d[:, ob, ts], ysb[:, ob, :])
    nc.finalize()
    return nc


_CACHE = {}


def _get_kernel(C: int):
    ts = _tile_sizes_for(C)
    if ts not in _CACHE:
        _CACHE[ts] = _build_kernel(ts)
    return _CACHE[ts], sum(ts)


def _routing(x, router_w):
    """Replicate the reference's router bit-exactly with jax on CPU."""
    import jax
    import jax.numpy as jnp
    cpu = jax.devices("cpu")[0]
    with jax.default_device(cpu):
        xf = jnp.asarray(np.asarray(x)).reshape(-1, H)
        rw = jnp.asarray(np.asarray(router_w))
        logits = xf @ rw.T
        probs = jax.nn.softmax(logits, axis=-1)
        top_v, top_i = jax.lax.top_k(probs, KTOP)
        top_v = top_v / jnp.sum(top_v, axis=-1, keepdims=True)
        usage = probs.mean(axis=0)
        lb_loss = np.float32(0.01) * (E * jnp.sum(usage * usage))
    return (np.asarray(top_i), np.asarray(top_v),
            np.asarray(lb_loss, dtype=np.float32))


def kernel(x, router_w, gate_w, up_w, down_w, _trace=False):
    x = np.ascontiguousarray(np.asarray(x), dtype=np.float32)
    router_w = np.asarray(router_w, dtype=np.float32)
    gate_w = np.asarray(gate_w, dtype=np.float32)
    up_w = np.asarray(up_w, dtype=np.float32)
    down_w = np.asarray(down_w, dtype=np.float32)
    b, t_, h = x.shape
    n = b * t_
    xf = x.reshape(n, h)

    top_i, top_v, lb_loss = _routing(x, router_w)

    # dispatch: token indices + gate values per expert
    idxs, gates = [], []
    for e in range(E):
        sel = (top_i == e)
        tok = np.nonzero(sel.any(axis=1))[0]
        gv = top_v[tok, np.argmax(sel[tok], axis=1)] if len(tok) else np.zeros(0)
        idxs.append(tok)
        gates.append(gv.astype(np.float32))
    counts = [len(i) for i in idxs]
    C = max(512, -(-max(counts) // 128) * 128)

    nc, C = _get_kernel(C)

    def to_pf(a, nb):  # [nb*128, cols] -> [128, nb, cols]
        return np.ascontiguousarray(
            a.reshape(nb, 128, a.shape[1]).transpose(1, 0, 2), dtype=np.float32)

    in_maps = []
    for e in range(E):
        xg = np.zeros((C, H), dtype=np.float32)
        xg[:counts[e]] = xf[idxs[e]]
        in_maps.append({
            "xt": np.ascontiguousarray(xg.reshape(C, HB, 128).transpose(2, 1, 0)),
            "wg": to_pf(gate_w[e].T, HB),
            "wu": to_pf(up_w[e].T, HB),
            "wd": to_pf(down_w[e].T, FB),
        })

    res = bass_utils.run_bass_kernel_spmd(
        nc, in_maps, core_ids=list(range(E)), trace=_trace)

    out = np.zeros((n, h), dtype=np.float32)
    for e in range(E):
        yt = res.results[e]["yt"]                        # [128, HB, C]
        y = yt.transpose(2, 1, 0).reshape(C, H)[:counts[e]]
        out[idxs[e]] += gates[e][:, None] * y

    kernel.last_results = res
    return out.reshape(b, t_, h), lb_loss
